# revision 2
# baseline (speedup 1.0000x reference)
"""Trainium2 Bass kernel for the rank-weighted log-loss reduction.

loss = -sum_i ri * (log(p_win_i) - R*(f0_i - P1)^2),  ri = i / (n*(n+1)/2)

Strategy (pure data parallel over 8 cores):
  - core k gets rows [k*M, (k+1)*M), M = N/8
  - on-chip per tile: Square on the scalar engine (bf16 out), p_win via an
    IN-PLACE predicated copy (f0 <- f1 where pv!=0), Ln (bf16 out), bf16
    subtract, then per-512-column-chunk matmuls [128,3] x [128,<=512]
    accumulate (sum per, sum w_lo*per, sum w_hi*per) into PSUM. The chunk
    weight columns encode the full in-core row base
    w(p) = row0_t + p*F_t + 512c  split at 256 granularity so every value
    is exact in bf16.
  - tile sizes taper (4096 -> 256 rows/partition) so bulk DMAs are 4 MB
    (best HBM efficiency) while the final serial compute chain is short.
  - two PSUM accumulators: the big one is copied out while the last two
    tiles still compute; only a [3,256] copy + 3 KB DMA trail the last
    input byte.
  - host folds the per-core [3,512]+[3,256] partials into the closed-form
    weighted sum in float64.
"""

import numpy as np
import ml_dtypes
from contextlib import ExitStack

import concourse.bass as bass
import concourse.mybir as mybir
import concourse.tile as tile
from concourse.bass_utils import run_bass_kernel_spmd


MAX_SYNC_WAITS = 1


def _spill_excess_waits(nc, max_waits=MAX_SYNC_WAITS):
    """The walrus in this toolchain rejects instructions carrying more than
    a couple of sync waits ("Too many sync wait commands"). Spill the excess
    onto same-engine NOPs inserted immediately before — semantically
    identical (consecutive sem-ge waits on one engine)."""
    import bass_rust

    k = 0
    for f in nc.m.functions:
        for b in f.blocks:
            out = []
            changed = False
            for inst in b.instructions:
                si = inst.sync_info
                waits = list(si.on_wait or []) if si is not None else []
                if len(waits) > max_waits:
                    chunks = [
                        waits[i : i + max_waits]
                        for i in range(0, len(waits), max_waits)
                    ]
                    for chunk in chunks[:-1]:
                        nop = mybir.InstNoOp(name=f"antspill-{k}", ins=[], outs=[])
                        k += 1
                        nop.engine = inst.engine
                        nop.sync_info = bass_rust.SyncInfo(
                            on_wait=chunk, on_update=[]
                        )
                        out.append(nop)
                    inst.sync_info = bass_rust.SyncInfo(
                        on_wait=chunks[-1], on_update=list(si.on_update or [])
                    )
                    changed = True
                out.append(inst)
            if changed:
                b.instructions = out
    return nc


N_TOTAL = 16777216
N_CORES = 8
P = 128            # SBUF partitions
M = N_TOTAL // N_CORES
P1 = 0.5

# rows-per-partition per tile; bulk tiles are 4 MB DMAs, tapering so the
# tail chain after the last input byte is short. Last N_B tiles go to the
# small PSUM accumulator B.
FS = [4096, 4096, 4096, 2048, 1024, 512, 256, 256]
N_B = 2
XMAX = max(FS)
assert sum(FS) * P == M


def _chunks(F):
    """(chunk_col_offset, width) pairs covering [0, F) in <=512 columns."""
    return [(512 * c, min(512, F - 512 * c)) for c in range((F + 511) // 512)]


def _sched():
    """Static schedule: per tile (F, row0, [(off, w), ...], use_acc_b)."""
    out = []
    row0 = 0
    for t, F in enumerate(FS):
        out.append((F, row0, _chunks(F), t >= len(FS) - N_B))
        row0 += P * F
    return out


NQ = sum(len(ch) for _, _, ch, _ in _sched())  # total chunk count


def build_nc():
    nc = bass.Bass(
        "TRN2", target_bir_lowering=False, debug=False,
        enable_asserts=False, num_devices=1,
    )
    fo = nc.dram_tensor("fo", [M, 2], mybir.dt.float32, kind="ExternalInput")
    pv = nc.dram_tensor("pv", [M], mybir.dt.int32, kind="ExternalInput")
    wt = nc.dram_tensor("wt", [P, 3 * NQ], mybir.dt.bfloat16, kind="ExternalInput")
    outa = nc.dram_tensor("outa", [3, 512], mybir.dt.float32, kind="ExternalOutput")
    outb = nc.dram_tensor("outb", [3, 256], mybir.dt.float32, kind="ExternalOutput")

    sched = _sched()
    fo_ap = fo.ap()
    pv_ap = pv.ap()

    with tile.TileContext(nc) as tc, ExitStack() as ctx:
        xp = ctx.enter_context(tc.tile_pool(name="xp", bufs=2))
        vp = ctx.enter_context(tc.tile_pool(name="vp", bufs=2))
        lp = ctx.enter_context(tc.tile_pool(name="lp", bufs=2))
        sp = ctx.enter_context(tc.tile_pool(name="sp", bufs=2))
        cp = ctx.enter_context(tc.tile_pool(name="cp", bufs=1))
        ps = ctx.enter_context(tc.tile_pool(name="ps", bufs=1, space="PSUM"))

        accA = ps.tile([3, 512], mybir.dt.float32, tag="accA")
        accB = ps.tile([3, 256], mybir.dt.float32, tag="accB")

        W = cp.tile([P, 3 * NQ], mybir.dt.bfloat16)
        nbias = cp.tile([P, 1], mybir.dt.float32)

        q = 0          # global chunk index
        started = {False: False, True: False}
        n_a_tiles = len(FS) - N_B
        for t, (F, row0, chunks, use_b) in enumerate(sched):
            X = xp.tile([P, XMAX, 2], mybir.dt.float32, tag="X")
            V = vp.tile([P, XMAX], mybir.dt.int32, tag="V")
            rows = P * F
            nc.sync.dma_start(
                X[:, :F, :],
                fo_ap[row0 : row0 + rows].rearrange("(p f) c -> p f c", p=P, f=F),
            )
            nc.sync.dma_start(
                V[:, :F],
                pv_ap[row0 : row0 + rows].rearrange("(p f) -> p f", p=P, f=F),
            )
            if t == 0:
                # constants load AFTER the first data DMAs are on the queue
                nc.sync.dma_start(W[:], wt[:])
                nc.vector.memset(nbias[:], -P1)

            S = sp.tile([P, XMAX], mybir.dt.bfloat16, tag="S")
            L = lp.tile([P, XMAX], mybir.dt.bfloat16, tag="L")
            # S = (f0 - 0.5)^2  (bf16 out; must read f0 before the in-place
            # select overwrites it)
            nc.scalar.activation(
                S[:, :F], X[:, :F, 0], mybir.ActivationFunctionType.Square,
                bias=nbias[:],
            )
            # f0 <- f1 where pv != 0 (in-place select on the strided view)
            nc.vector.copy_predicated(X[:, :F, 0], V[:, :F], X[:, :F, 1])
            # L = ln(p_win)  (bf16 out)
            nc.scalar.activation(
                L[:, :F], X[:, :F, 0], mybir.ActivationFunctionType.Ln
            )
            # per = L - S in bf16 (2x DVE rate)
            per = lp.tile([P, XMAX], mybir.dt.bfloat16, tag="per")
            nc.vector.tensor_sub(per[:, :F], L[:, :F], S[:, :F])

            acc = accB if use_b else accA
            for ci, (off, w) in enumerate(chunks):
                last = (t == n_a_tiles - 1 if not use_b else t == len(FS) - 1) \
                    and ci == len(chunks) - 1
                nc.tensor.matmul(
                    acc[:, :w], W[:, 3 * q : 3 * q + 3], per[:, off : off + w],
                    start=not started[use_b], stop=last,
                )
                started[use_b] = True
                q += 1

            if t == n_a_tiles - 1:
                # acc A complete: drain it while the B tiles compute
                obA = cp.tile([3, 512], mybir.dt.float32)
                nc.vector.tensor_copy(obA[:], accA[:])
                nc.sync.dma_start(outa[:], obA[:])

        obB = cp.tile([3, 256], mybir.dt.float32)
        nc.vector.tensor_copy(obB[:], accB[:])
        nc.sync.dma_start(outb[:], obB[:])
    _spill_excess_waits(nc)
    return nc


def build_wt():
    """Stationary weight columns per 512-column chunk: (ones, w_lo, w_hi)
    with w(p) = row0_t + p*F_t + off_c split at 256 granularity —
    w_lo = 256*(u & 255), w_hi = 65536*(u >> 8), u = w/256 < 8192 — so
    every column value is exact in bf16."""
    cols = np.zeros((P, 3 * NQ), np.float32)
    p_idx = np.arange(P, dtype=np.int64)
    q = 0
    for F, row0, chunks, _ in _sched():
        for off, _w in chunks:
            u = (row0 + p_idx * F + off) // 256
            assert u.max() < 8192
            cols[:, 3 * q] = 1.0
            cols[:, 3 * q + 1] = 256 * (u & 255)
            cols[:, 3 * q + 2] = 65536 * (u >> 8)
            q += 1
    out = cols.astype(ml_dtypes.bfloat16)
    assert np.all(out.astype(np.float32) == cols)
    return out


def combine(outs):
    """Fold per-core ([3,512], [3,256]) partials into the loss.

    Row i = k*M + w(p,chunk) + j with the w part already accumulated via
    the lo/hi columns; j is the within-chunk column index.
    """
    n = M * len(outs)
    # mirror the reference's fp32 denom computation
    denom = float(np.float32(n) * np.float32(n + 1) * np.float32(0.5))
    jA = np.arange(512, dtype=np.float64)
    jB = np.arange(256, dtype=np.float64)
    total = 0.0
    for k, (oa, ob) in enumerate(outs):
        a = oa.astype(np.float64)
        b = ob.astype(np.float64)
        total += (k * M) * (a[0].sum() + b[0].sum())
        total += a[1].sum() + a[2].sum() + b[1].sum() + b[2].sum()
        total += (jA * a[0]).sum() + (jB * b[0]).sum()
    return -total / denom


_NC_CACHE = {}


def _run(final_out, point_victor, **spmd_kwargs):
    fo = np.ascontiguousarray(np.asarray(final_out, dtype=np.float32))
    pv = np.ascontiguousarray(np.asarray(point_victor, dtype=np.int32))
    assert fo.shape == (N_TOTAL, 2) and pv.shape == (N_TOTAL,)

    if "nc" not in _NC_CACHE:
        _NC_CACHE["nc"] = build_nc()
    nc = _NC_CACHE["nc"]
    wt = build_wt()

    in_maps = [
        {"fo": fo[k * M : (k + 1) * M], "pv": pv[k * M : (k + 1) * M], "wt": wt}
        for k in range(N_CORES)
    ]
    res = run_bass_kernel_spmd(nc, in_maps, core_ids=list(range(N_CORES)), **spmd_kwargs)
    outs = [(r["outa"], r["outb"]) for r in res.results]
    return np.float32(combine(outs)), res


def kernel(final_out, point_victor):
    return _run(final_out, point_victor)[0]


# revision 3
# speedup vs baseline: 1.3436x; 1.3436x over previous
"""Trainium2 Bass kernel for the rank-weighted log-loss reduction.

loss = -sum_i ri * (log(p_win_i) - R*(f0_i - P1)^2),  ri = i / (n*(n+1)/2)

Strategy (pure data parallel over 8 cores):
  - core k gets rows [k*M, (k+1)*M), M = N/8
  - on-chip per tile: DVE copies f0 into a contiguous pw buffer and
    predicated-copies f1 over it where pv!=0; ACT computes
    S=(f0-0.5)^2 and L=ln(pw), both straight to bf16; per 512-column
    chunk, TWO matmuls [128,3] x [128,<=512] with +W (on L) and -W (on S)
    accumulate (sum per, sum w_lo*per, sum w_hi*per) into PSUM — the
    subtract lives in the matmul sign, keeping DVE off the critical path.
  - chunk weight columns encode the full in-core row base
    w(p) = row0_t + p*F_t + off_c, split at 256 granularity so every
    value is exact in bf16.
  - tile sizes taper (2048 -> 256 rows/partition) with 3-deep buffering
    so the DMA queue never stalls on slot release and the final serial
    chain after the last input byte is short.
  - two PSUM accumulators: the big one drains while the last two tiles
    compute; output DMAs are emitted after all input DMAs so the
    in-order sync queue never blocks input transfers.
  - host folds the per-core [3,512]+[3,256] partials into the
    closed-form weighted sum in float64.
"""

import numpy as np
import ml_dtypes
from contextlib import ExitStack

import concourse.bass as bass
import concourse.mybir as mybir
import concourse.tile as tile
from concourse.bass_utils import run_bass_kernel_spmd


MAX_SYNC_WAITS = 1


def _spill_excess_waits(nc, max_waits=MAX_SYNC_WAITS):
    """The walrus in this toolchain rejects instructions carrying more than
    a couple of sync waits ("Too many sync wait commands"). Spill the excess
    onto same-engine NOPs inserted immediately before — semantically
    identical (consecutive sem-ge waits on one engine)."""
    import bass_rust

    k = 0
    for f in nc.m.functions:
        for b in f.blocks:
            out = []
            changed = False
            for inst in b.instructions:
                si = inst.sync_info
                waits = list(si.on_wait or []) if si is not None else []
                if len(waits) > max_waits:
                    chunks = [
                        waits[i : i + max_waits]
                        for i in range(0, len(waits), max_waits)
                    ]
                    for chunk in chunks[:-1]:
                        nop = mybir.InstNoOp(name=f"antspill-{k}", ins=[], outs=[])
                        k += 1
                        nop.engine = inst.engine
                        nop.sync_info = bass_rust.SyncInfo(
                            on_wait=chunk, on_update=[]
                        )
                        out.append(nop)
                    inst.sync_info = bass_rust.SyncInfo(
                        on_wait=chunks[-1], on_update=list(si.on_update or [])
                    )
                    changed = True
                out.append(inst)
            if changed:
                b.instructions = out
    return nc


N_TOTAL = 16777216
N_CORES = 8
P = 128            # SBUF partitions
M = N_TOTAL // N_CORES
P1 = 0.5

# rows-per-partition per tile; bulk tiles are 2 MB DMAs, tapering so the
# tail chain after the last input byte is short. Last N_B tiles go to the
# small PSUM accumulator B.
FS = [2048] * 7 + [1024, 512, 256, 256]
N_B = 2
XMAX = max(FS)
assert sum(FS) * P == M


def _chunks(F):
    """(chunk_col_offset, width) pairs covering [0, F) in <=512 columns."""
    return [(512 * c, min(512, F - 512 * c)) for c in range((F + 511) // 512)]


def _sched():
    """Static schedule: per tile (F, row0, [(off, w), ...], use_acc_b)."""
    out = []
    row0 = 0
    for t, F in enumerate(FS):
        out.append((F, row0, _chunks(F), t >= len(FS) - N_B))
        row0 += P * F
    return out


NQ = sum(len(ch) for _, _, ch, _ in _sched())  # total chunk count


def build_nc():
    nc = bass.Bass(
        "TRN2", target_bir_lowering=False, debug=False,
        enable_asserts=False, num_devices=1,
    )
    fo = nc.dram_tensor("fo", [M, 2], mybir.dt.float32, kind="ExternalInput")
    pv = nc.dram_tensor("pv", [M], mybir.dt.int32, kind="ExternalInput")
    # per chunk q: cols [6q,6q+3) = +(1, w_lo, w_hi); [6q+3,6q+6) = -(...)
    wt = nc.dram_tensor("wt", [P, 6 * NQ], mybir.dt.bfloat16, kind="ExternalInput")
    outa = nc.dram_tensor("outa", [3, 512], mybir.dt.float32, kind="ExternalOutput")
    outb = nc.dram_tensor("outb", [3, 256], mybir.dt.float32, kind="ExternalOutput")

    sched = _sched()
    fo_ap = fo.ap()
    pv_ap = pv.ap()
    n_a_tiles = len(FS) - N_B

    with tile.TileContext(nc) as tc, ExitStack() as ctx:
        xp = ctx.enter_context(tc.tile_pool(name="xp", bufs=3))
        vp = ctx.enter_context(tc.tile_pool(name="vp", bufs=3))
        pp = ctx.enter_context(tc.tile_pool(name="pp", bufs=3))
        lp = ctx.enter_context(tc.tile_pool(name="lp", bufs=3))
        sp = ctx.enter_context(tc.tile_pool(name="sp", bufs=3))
        cp = ctx.enter_context(tc.tile_pool(name="cp", bufs=1))
        ps = ctx.enter_context(tc.tile_pool(name="ps", bufs=1, space="PSUM"))

        accA = ps.tile([3, 512], mybir.dt.float32, tag="accA")
        accB = ps.tile([3, 256], mybir.dt.float32, tag="accB")

        W = cp.tile([P, 6 * NQ], mybir.dt.bfloat16)
        nbias = cp.tile([P, 1], mybir.dt.float32)

        q = 0          # global chunk index
        started = {False: False, True: False}
        for t, (F, row0, chunks, use_b) in enumerate(sched):
            X = xp.tile([P, XMAX, 2], mybir.dt.float32, tag="X")
            V = vp.tile([P, XMAX], mybir.dt.int32, tag="V")
            rows = P * F
            nc.sync.dma_start(
                X[:, :F, :],
                fo_ap[row0 : row0 + rows].rearrange("(p f) c -> p f c", p=P, f=F),
            )
            nc.sync.dma_start(
                V[:, :F],
                pv_ap[row0 : row0 + rows].rearrange("(p f) -> p f", p=P, f=F),
            )
            if t == 0:
                # constants load AFTER the first data DMAs are on the queue
                nc.sync.dma_start(W[:], wt[:])
                nc.vector.memset(nbias[:], -P1)

            pw = pp.tile([P, XMAX], mybir.dt.float32, tag="pw")
            S = sp.tile([P, XMAX], mybir.dt.bfloat16, tag="S")
            L = lp.tile([P, XMAX], mybir.dt.bfloat16, tag="L")
            # pw = f0 (contiguous), then f1 where pv != 0
            nc.vector.tensor_copy(pw[:, :F], X[:, :F, 0])
            # S = (f0 - 0.5)^2  (bf16 out, strided read; independent of pw)
            nc.scalar.activation(
                S[:, :F], X[:, :F, 0], mybir.ActivationFunctionType.Square,
                bias=nbias[:],
            )
            nc.vector.copy_predicated(pw[:, :F], V[:, :F], X[:, :F, 1])
            # L = ln(p_win)  (bf16 out)
            nc.scalar.activation(
                L[:, :F], pw[:, :F], mybir.ActivationFunctionType.Ln
            )

            acc = accB if use_b else accA
            lastt = t == (len(FS) - 1 if use_b else n_a_tiles - 1)
            for ci, (off, w) in enumerate(chunks):
                last = lastt and ci == len(chunks) - 1
                nc.tensor.matmul(
                    acc[:, :w], W[:, 6 * q : 6 * q + 3], L[:, off : off + w],
                    start=not started[use_b], stop=False,
                )
                nc.tensor.matmul(
                    acc[:, :w], W[:, 6 * q + 3 : 6 * q + 6], S[:, off : off + w],
                    start=False, stop=last,
                )
                started[use_b] = True
                q += 1

        # drains emitted after every input DMA: the in-order sync queue
        # never stalls input transfers on an output dependency. acc A's
        # copy still executes as soon as its stop-matmul lands.
        obA = cp.tile([3, 512], mybir.dt.float32)
        nc.vector.tensor_copy(obA[:], accA[:])
        nc.sync.dma_start(outa[:], obA[:])
        obB = cp.tile([3, 256], mybir.dt.float32)
        nc.vector.tensor_copy(obB[:], accB[:])
        nc.sync.dma_start(outb[:], obB[:])
    _spill_excess_waits(nc)
    return nc


def build_wt():
    """Stationary weight columns per 512-column chunk: +/-(ones, w_lo, w_hi)
    with w(p) = row0_t + p*F_t + off_c split at 256 granularity —
    w_lo = 256*(u & 255), w_hi = 65536*(u >> 8), u = w/256 < 8192 — so
    every column value is exact in bf16. The negated triple applies the
    subtraction of the square term inside the PSUM accumulation."""
    cols = np.zeros((P, 6 * NQ), np.float32)
    p_idx = np.arange(P, dtype=np.int64)
    q = 0
    for F, row0, chunks, _ in _sched():
        for off, _w in chunks:
            u = (row0 + p_idx * F + off) // 256
            assert u.max() < 8192
            w3 = np.stack(
                [np.ones(P, np.float32), (256 * (u & 255)).astype(np.float32),
                 (65536 * (u >> 8)).astype(np.float32)], axis=1
            )
            cols[:, 6 * q : 6 * q + 3] = w3
            cols[:, 6 * q + 3 : 6 * q + 6] = -w3
            q += 1
    out = cols.astype(ml_dtypes.bfloat16)
    assert np.all(out.astype(np.float32) == cols)
    return out


def combine(outs):
    """Fold per-core ([3,512], [3,256]) partials into the loss.

    Row i = k*M + w(p,chunk) + j with the w part already accumulated via
    the lo/hi columns; j is the within-chunk column index.
    """
    n = M * len(outs)
    # mirror the reference's fp32 denom computation
    denom = float(np.float32(n) * np.float32(n + 1) * np.float32(0.5))
    jA = np.arange(512, dtype=np.float64)
    jB = np.arange(256, dtype=np.float64)
    total = 0.0
    for k, (oa, ob) in enumerate(outs):
        a = oa.astype(np.float64)
        b = ob.astype(np.float64)
        total += (k * M) * (a[0].sum() + b[0].sum())
        total += a[1].sum() + a[2].sum() + b[1].sum() + b[2].sum()
        total += (jA * a[0]).sum() + (jB * b[0]).sum()
    return -total / denom


_NC_CACHE = {}


def _run(final_out, point_victor, **spmd_kwargs):
    fo = np.ascontiguousarray(np.asarray(final_out, dtype=np.float32))
    pv = np.ascontiguousarray(np.asarray(point_victor, dtype=np.int32))
    assert fo.shape == (N_TOTAL, 2) and pv.shape == (N_TOTAL,)

    if "nc" not in _NC_CACHE:
        _NC_CACHE["nc"] = build_nc()
    nc = _NC_CACHE["nc"]
    wt = build_wt()

    in_maps = [
        {"fo": fo[k * M : (k + 1) * M], "pv": pv[k * M : (k + 1) * M], "wt": wt}
        for k in range(N_CORES)
    ]
    res = run_bass_kernel_spmd(nc, in_maps, core_ids=list(range(N_CORES)), **spmd_kwargs)
    outs = [(r["outa"], r["outb"]) for r in res.results]
    return np.float32(combine(outs)), res


def kernel(final_out, point_victor):
    return _run(final_out, point_victor)[0]
